# revision 9
# baseline (speedup 1.0000x reference)
"""Trainium2 Bass kernel for the CNN/segment-reduce model.

Strategy (pure data-parallel over batch, 8 cores x 64 batch elems):
  host:   gather pos embeddings, transpose/concat/zero-pad the conv input to
          [ci=1024, l=128] per batch elem (3 zero cols each side for 'same'
          conv padding up to k=7), precompute segment weight-masks m/cnt,
          reorder fc_w columns to the on-device feature layout (fc_b folded
          in via a constant-1 feature), convert PE-facing data to bf16.
  device: conv = PE matmuls, contraction over 8 ci-chunks x k taps with the
          [128ci,128co] weight block stationary; rhs = shifted x window over
          4 batch elems (N=512); accumulate in PSUM [128co, 4x128l].
          ACT tanh(+bias) -> bf16 SBUF; DVE mask-multiply + segment-reduce
          -> feature tile [128, 23*64]; FC = 23 accumulating matmuls into
          PSUM [64b, 19]; fused exp/sum softmax; DMA out fp32.
"""

import numpy as np
import ml_dtypes

B, S, DW, DP, DC, VP, VR = 512, 128, 300, 50, 256, 256, 19
KS = (3, 5, 7)
CIN = 3 * DW + 2 * DP  # 1000
CINP = 1024            # padded with zero channels
NCH = CINP // 128      # 8 contraction chunks
NCORE = 8
BC = B // NCORE        # 64 batch elems per core
NB = 16                # batch elems per resident x group
NBG = BC // NB         # 4 groups
NQ = NB // 4           # 4 psum quads (4 b per N=512 matmul)
LW = S + 6             # 3 zero cols each side
PAD = 3
SL = S - 1             # conv cols actually consumed (l=127 never pooled)
NF = 4 * SL            # matmul free size (4 batch elems)
NSETS = sum(k * NCH for k in KS) * 2          # 240 weight blocks
NCHK = 23                                     # feature chunks of 128
F_CONV = 18 * 128                             # 2304 conv features
BF16 = ml_dtypes.bfloat16

# weight block index: ordered (ki-major, h, t, c)
_PREFIX = {}
_off = 0
for _ki, _k in enumerate(KS):
    for _h in range(2):
        _PREFIX[(_ki, _h)] = _off
        _off += _k * NCH


def _bidx(ki, h, t, c):
    return _PREFIX[(ki, h)] + t * NCH + c


def _build_program():
    from contextlib import ExitStack
    import concourse.tile as tile
    from concourse import bacc, mybir

    f32 = mybir.dt.float32
    bf16 = mybir.dt.bfloat16
    AF = mybir.ActivationFunctionType
    ALU = mybir.AluOpType
    AX = mybir.AxisListType

    nc = bacc.Bacc("TRN2", target_bir_lowering=False, debug=False,
                   num_devices=NCORE)

    X = nc.declare_dram_parameter("X", [NBG, NCH, 128, NB * LW], bf16,
                                  isOutput=False)
    WM = nc.declare_dram_parameter("WM", [NBG, 128, 3 * NB * SL], bf16,
                                   isOutput=False)
    WT = nc.declare_dram_parameter("WT", [128, NSETS * 128], bf16,
                                   isOutput=False)
    FCW = nc.declare_dram_parameter("FCW", [128, NCHK * VR], bf16,
                                    isOutput=False)
    E12 = nc.declare_dram_parameter("E12", [5 * 128, BC], f32, isOutput=False)
    CB = nc.declare_dram_parameter("CB", [128, 6], f32, isOutput=False)
    OUT = nc.declare_dram_parameter("OUT", [BC, VR], f32, isOutput=True)

    with tile.TileContext(nc) as tc, ExitStack() as ctx:
        const = ctx.enter_context(tc.tile_pool(name="const", bufs=1))
        xpool = ctx.enter_context(tc.tile_pool(name="xp", bufs=2))
        mpool = ctx.enter_context(tc.tile_pool(name="mp", bufs=2))
        tpool = ctx.enter_context(tc.tile_pool(name="tp", bufs=4))
        prpool = ctx.enter_context(tc.tile_pool(name="prp", bufs=4))
        pspool = ctx.enter_context(tc.tile_pool(name="psp", bufs=8,
                                                space="PSUM"))

        # PE warm-up: dummy matmuls with no DMA deps so the HAM clock-gate
        # flips to 8/8 while the first weight/x DMAs are in flight.
        warm = const.tile([128, NF], bf16)
        nc.any.memset(warm[:], 0.0)
        wps = pspool.tile([128, NF], f32, tag="ps")
        for _ in range(10):
            nc.tensor.matmul(wps[:], warm[:, 0:128], warm[:],
                             start=True, stop=True)

        wt_sb = const.tile([128, NSETS * 128], bf16)
        # first conv group's weights first, then everything else
        nc.sync.dma_start(wt_sb[:, 0:_PREFIX[(0, 1)] * 128],
                          WT.ap()[:, 0:_PREFIX[(0, 1)] * 128])
        fcw_sb = const.tile([128, NCHK * VR], bf16)
        cb_sb = const.tile([128, 6], f32)
        nc.sync.dma_start(cb_sb[:], CB.ap()[:])
        feat32 = const.tile([128, NCHK * BC], f32)
        featbf = const.tile([128, NCHK * BC], bf16)

        fcps = pspool.tile([BC, VR], f32, tag="ps")
        for bg in range(NBG):
            x_sb = xpool.tile([128, NCH * NB * LW], bf16, tag="x",
                              name=f"x_sb_{bg}")
            for c in range(NCH):
                nc.sync.dma_start(x_sb[:, c * NB * LW:(c + 1) * NB * LW],
                                  X.ap()[bg, c])
            wm_sb = mpool.tile([128, 3 * NB * SL], bf16, tag="wm",
                               name=f"wm_sb_{bg}")
            nc.sync.dma_start(wm_sb[:], WM.ap()[bg])
            if bg == 0:
                # stream the rest of the weights in behind x/wm for bg0
                nc.sync.dma_start(
                    wt_sb[:, _PREFIX[(0, 1)] * 128:],
                    WT.ap()[:, _PREFIX[(0, 1)] * 128:])
                nc.sync.dma_start(fcw_sb[:], FCW.ap()[:])
                for j in range(5):
                    nc.sync.dma_start(
                        feat32[:, (18 + j) * BC:(19 + j) * BC],
                        E12.ap()[j * 128:(j + 1) * 128, :])
            xv = x_sb[:].rearrange("p (c b w) -> p c b w", c=NCH, b=NB)

            for ki, k in enumerate(KS):
                for h in range(2):
                    pss = [pspool.tile([128, NF], f32, tag="ps",
                                       name=f"ps_{bg}_{ki}_{h}_{q}")
                           for q in range(NQ)]
                    nsets = k * NCH
                    si = 0
                    for t in range(k):
                        s = t - k // 2
                        for c in range(NCH):
                            wblk = wt_sb[:, _bidx(ki, h, t, c) * 128:
                                         (_bidx(ki, h, t, c) + 1) * 128]
                            for q in range(NQ):
                                rhs = xv[:, c, q * 4:(q + 1) * 4,
                                         PAD + s:PAD + s + SL]
                                nc.tensor.matmul(pss[q][:], wblk, rhs,
                                                 start=(si == 0),
                                                 stop=(si == nsets - 1))
                            si += 1
                    for q in range(NQ):
                        th = tpool.tile([128, NF], bf16, tag="th",
                                        name=f"th_{bg}_{ki}_{h}_{q}")
                        nc.scalar.activation(
                            th[:], pss[q][:], AF.Tanh,
                            bias=cb_sb[:, ki * 2 + h:ki * 2 + h + 1])
                        for seg in range(3):
                            pr = prpool.tile([128, NF], bf16, tag="pr",
                                             name=f"pr_{bg}_{ki}_{h}_{q}_{seg}")
                            nc.vector.tensor_tensor(
                                pr[:], th[:],
                                wm_sb[:, seg * NB * SL + q * NF:
                                      seg * NB * SL + (q + 1) * NF],
                                op=ALU.mult)
                            ch = ki * 6 + seg * 2 + h
                            col0 = ch * BC + bg * NB + q * 4
                            nc.vector.tensor_reduce(
                                feat32[:, col0:col0 + 4],
                                pr[:].rearrange("p (b w) -> p b w", w=SL),
                                axis=AX.X, op=ALU.add)

            if bg % 2 == 1:
                # half the batch is fully pooled: cast + FC for b-range
                half = bg // 2
                b0 = half * 32
                nc.vector.tensor_copy(
                    featbf[:].rearrange("p (c b) -> p c b",
                                        c=NCHK)[:, :, b0:b0 + 32],
                    feat32[:].rearrange("p (c b) -> p c b",
                                        c=NCHK)[:, :, b0:b0 + 32])
                for ch in range(NCHK):
                    nc.tensor.matmul(
                        fcps[b0:b0 + 32, :],
                        featbf[:, ch * BC + b0:ch * BC + b0 + 32],
                        fcw_sb[:, ch * VR:(ch + 1) * VR],
                        start=(ch == 0), stop=(ch == NCHK - 1))
        mx = const.tile([BC, 1], f32)
        nc.vector.tensor_reduce(mx[:], fcps[:], axis=AX.X, op=ALU.max,
                                negate=True)
        esm = const.tile([BC, VR], f32)
        ssum = const.tile([BC, 1], f32)
        nc.scalar.activation(esm[:], fcps[:], AF.Exp, bias=mx[:],
                             accum_out=ssum[:])
        rin = const.tile([BC, 1], f32)
        nc.vector.reciprocal(rin[:], ssum[:])
        osb = const.tile([BC, VR], f32)
        nc.vector.tensor_scalar_mul(osb[:], esm[:], rin[:])
        nc.sync.dma_start(OUT.ap()[:], osb[:])

    nc.compile()
    return nc


_NC_CACHE = []


def _get_program():
    if not _NC_CACHE:
        _NC_CACHE.append(_build_program())
    return _NC_CACHE[0]


def _prep_inputs(W, e1, e2, pos_emb1, pos_emb2, conv_ws, conv_bs, fc_w, fc_b,
                 W_pos1, W_pos2, e1_p, e2_p):
    """Host-side data layout; returns per-core input maps."""
    # --- conv input: [B, 1024, 128] zero-padded, 3 zero cols each side ---
    Wp1 = pos_emb1[W_pos1]          # [B, S, DP]
    Wp2 = pos_emb2[W_pos2]
    Xf = np.concatenate([W, Wp1, Wp2], axis=2).transpose(0, 2, 1)  # [B,CIN,S]
    Xpad = np.zeros((B, CINP, LW), np.float32)
    Xpad[:, :CIN, PAD:PAD + S] = Xf
    Xpad = Xpad.astype(BF16).reshape(NCORE, NBG, NB, NCH, 128, LW)
    Xc = np.ascontiguousarray(Xpad.transpose(0, 1, 3, 4, 2, 5)).reshape(
        NCORE, NBG, NCH, 128, NB * LW)

    # --- segment weight masks m/cnt, replicated over 128 partitions ---
    d1 = np.minimum(e1_p, e2_p).astype(np.int64)
    d2 = np.maximum(e1_p, e2_p).astype(np.int64)
    idx = np.arange(S)[None, :]
    m1 = (idx < d1[:, None])
    m2 = (idx >= d1[:, None]) & (idx < d2[:, None])
    m3 = (idx >= d2[:, None]) & (idx < S - 1)
    wm = np.stack([m1, m2, m3], axis=1).astype(np.float32)  # [B,3,S]
    cnt = np.maximum(wm.sum(axis=2), 1.0)
    wm /= cnt[:, :, None]
    wm = wm[:, :, :SL]  # l=127 is never pooled
    wm = wm.astype(BF16).reshape(NCORE, NBG, NB, 3, SL)
    wm = np.ascontiguousarray(wm.transpose(0, 1, 3, 2, 4)).reshape(
        NCORE, NBG, 1, 3 * NB * SL)
    WMc = np.ascontiguousarray(np.broadcast_to(
        wm, (NCORE, NBG, 128, 3 * NB * SL)))

    # --- conv weights -> stationary blocks [128ci, 128co], bf16 ---
    wt = np.zeros((128, NSETS * 128), np.float32)
    for ki, k in enumerate(KS):
        cw = np.zeros((DC, CINP, k), np.float32)
        cw[:, :CIN, :] = conv_ws[ki]
        for h in range(2):
            for t in range(k):
                for c in range(NCH):
                    blk = cw[h * 128:(h + 1) * 128,
                             c * 128:(c + 1) * 128, t]  # [co, ci]
                    wt[:, _bidx(ki, h, t, c) * 128:
                       (_bidx(ki, h, t, c) + 1) * 128] = blk.T
    wt = wt.astype(BF16)

    # --- fc weights in device feature order; fc_b via constant-1 feature ---
    # f' in [0, 2304): ch = ki*6+seg*2+h, p = co_local
    #   orig col = 600 + ki*768 + (h*128+p)*3 + seg
    # f' in [2304, 2904): orig col = f' - 2304   (e1, e2)
    # f' == 2904: constant-1 -> fc_b
    fcw = np.zeros((NCHK * 128, VR), np.float32)
    fp = np.arange(F_CONV)
    ch = fp // 128
    p = fp % 128
    ki = ch // 6
    seg = (ch % 6) // 2
    h = ch % 2
    orig = 600 + ki * 768 + (h * 128 + p) * 3 + seg
    fcw[fp] = fc_w[:, orig].T
    fcw[F_CONV:F_CONV + 600] = fc_w[:, :600].T
    fcw[F_CONV + 600] = fc_b
    fcw_host = np.ascontiguousarray(
        fcw.reshape(NCHK, 128, VR).transpose(1, 0, 2)).reshape(
        128, NCHK * VR).astype(BF16)

    # --- e1/e2 + constant-1 features, fp32, per core [640, BC] ---
    e12 = np.zeros((B, 5 * 128), np.float32)
    e12[:, :300] = e1
    e12[:, 300:600] = e2
    e12[:, 600] = 1.0
    E12c = np.ascontiguousarray(
        e12.reshape(NCORE, BC, 5 * 128).transpose(0, 2, 1))

    # --- conv biases [128, 6] fp32 ---
    cb = np.zeros((128, 6), np.float32)
    for ki in range(3):
        for h in range(2):
            cb[:, ki * 2 + h] = conv_bs[ki][h * 128:(h + 1) * 128]

    in_maps = []
    for i in range(NCORE):
        in_maps.append({
            "X": Xc[i], "WM": WMc[i], "WT": wt, "FCW": fcw_host,
            "E12": E12c[i], "CB": cb,
        })
    return in_maps


def kernel(**inputs):
    f = {k: np.asarray(v) for k, v in inputs.items()}
    in_maps = _prep_inputs(
        f["W"].astype(np.float32), f["e1"].astype(np.float32),
        f["e2"].astype(np.float32), f["pos_emb1"].astype(np.float32),
        f["pos_emb2"].astype(np.float32),
        [f["conv_w3"], f["conv_w5"], f["conv_w7"]],
        [f["conv_b3"], f["conv_b5"], f["conv_b7"]],
        f["fc_w"].astype(np.float32), f["fc_b"].astype(np.float32),
        f["W_pos1"], f["W_pos2"], f["e1_p"], f["e2_p"])

    from concourse.bass_utils import run_bass_kernel_spmd
    nc = _get_program()
    try:
        res = run_bass_kernel_spmd(nc, in_maps, core_ids=list(range(NCORE)))
    except Exception:
        # transient device wedge (e.g. NRT_EXEC_UNIT_UNRECOVERABLE from a
        # prior crashed process) usually clears on retry
        res = run_bass_kernel_spmd(nc, in_maps, core_ids=list(range(NCORE)))
    out = np.concatenate([res.results[i]["OUT"] for i in range(NCORE)],
                         axis=0)
    return out.astype(np.float32)


# revision 12
# speedup vs baseline: 1.0347x; 1.0347x over previous
"""Trainium2 Bass kernel for the CNN/segment-reduce model.

Strategy (pure data-parallel over batch, 8 cores x 64 batch elems):
  host:   gather pos embeddings, transpose/concat/zero-pad the conv input to
          [ci=1024, l=128] per batch elem (3 zero cols each side for 'same'
          conv padding up to k=7), precompute segment weight-masks m/cnt,
          reorder fc_w columns to the on-device feature layout (fc_b folded
          in via a constant-1 feature), convert PE-facing data to bf16.
  device: conv = PE matmuls, contraction over 8 ci-chunks x k taps with the
          [128ci,128co] weight block stationary; rhs = shifted x window over
          4 batch elems (N=512); accumulate in PSUM [128co, 4x128l].
          ACT tanh(+bias) -> bf16 SBUF; DVE mask-multiply + segment-reduce
          -> feature tile [128, 23*64]; FC = 23 accumulating matmuls into
          PSUM [64b, 19]; fused exp/sum softmax; DMA out fp32.
"""

import numpy as np
import ml_dtypes

B, S, DW, DP, DC, VP, VR = 512, 128, 300, 50, 256, 256, 19
KS = (3, 5, 7)
CIN = 3 * DW + 2 * DP  # 1000
CINP = 1024            # padded with zero channels
NCH = CINP // 128      # 8 contraction chunks
NCORE = 8
BC = B // NCORE        # 64 batch elems per core
NB = 16                # batch elems per resident x group
NBG = BC // NB         # 4 groups
NQ = NB // 4           # 4 psum quads (4 b per N=512 matmul)
LW = S + 6             # 3 zero cols each side
PAD = 3
SL = S - 1             # conv cols actually consumed (l=127 never pooled)
NF = 4 * SL            # matmul free size (4 batch elems)
NSETS = sum(k * NCH for k in KS) * 2          # 240 weight blocks
NCHK = 23                                     # feature chunks of 128
F_CONV = 18 * 128                             # 2304 conv features
BF16 = ml_dtypes.bfloat16

# weight block index: ordered (ki-major, h, t, c)
_PREFIX = {}
_off = 0
for _ki, _k in enumerate(KS):
    for _h in range(2):
        _PREFIX[(_ki, _h)] = _off
        _off += _k * NCH


def _bidx(ki, h, t, c):
    return _PREFIX[(ki, h)] + t * NCH + c


def _build_program():
    from contextlib import ExitStack
    import concourse.tile as tile
    from concourse import bacc, mybir

    f32 = mybir.dt.float32
    bf16 = mybir.dt.bfloat16
    AF = mybir.ActivationFunctionType
    ALU = mybir.AluOpType
    AX = mybir.AxisListType

    nc = bacc.Bacc("TRN2", target_bir_lowering=False, debug=False,
                   num_devices=NCORE)

    X = nc.declare_dram_parameter("X", [NBG, NCH, 128, NB * LW], bf16,
                                  isOutput=False)
    WM = nc.declare_dram_parameter("WM", [NBG, 128, 3 * NB * SL], bf16,
                                   isOutput=False)
    WT = nc.declare_dram_parameter("WT", [128, NSETS * 128], bf16,
                                   isOutput=False)
    FCW = nc.declare_dram_parameter("FCW", [128, NCHK * VR], bf16,
                                    isOutput=False)
    E12 = nc.declare_dram_parameter("E12", [5 * 128, BC], f32, isOutput=False)
    CB = nc.declare_dram_parameter("CB", [128, 6], f32, isOutput=False)
    OUT = nc.declare_dram_parameter("OUT", [BC, VR], f32, isOutput=True)

    with tile.TileContext(nc) as tc, ExitStack() as ctx:
        const = ctx.enter_context(tc.tile_pool(name="const", bufs=1))
        xpool = ctx.enter_context(tc.tile_pool(name="xp", bufs=2))
        mpool = ctx.enter_context(tc.tile_pool(name="mp", bufs=2))
        tpool = ctx.enter_context(tc.tile_pool(name="tp", bufs=4))
        prpool = ctx.enter_context(tc.tile_pool(name="prp", bufs=4))
        pspool = ctx.enter_context(tc.tile_pool(name="psp", bufs=8,
                                                space="PSUM"))

        # PE warm-up: dummy matmuls with no DMA deps so the HAM clock-gate
        # flips to 8/8 while the first weight/x DMAs are in flight.
        warm = const.tile([128, NF], bf16)
        nc.any.memset(warm[:], 0.0)
        wps = pspool.tile([128, NF], f32, tag="ps")
        for _ in range(10):
            nc.tensor.matmul(wps[:], warm[:, 0:128], warm[:],
                             start=True, stop=True)

        wt_sb = const.tile([128, NSETS * 128], bf16)
        # one DMA per (ki,h) group so each group unblocks as its slice lands
        wt_bounds = sorted(_PREFIX.values()) + [NSETS]
        nc.sync.dma_start(wt_sb[:, 0:wt_bounds[1] * 128],
                          WT.ap()[:, 0:wt_bounds[1] * 128])
        fcw_sb = const.tile([128, NCHK * VR], bf16)
        cb_sb = const.tile([128, 6], f32)
        nc.sync.dma_start(cb_sb[:], CB.ap()[:])
        feat32 = const.tile([128, NCHK * BC], f32)
        featbf = const.tile([128, NCHK * BC], bf16)

        fcps = pspool.tile([BC, VR], f32, tag="ps")
        for bg in range(NBG):
            x_sb = xpool.tile([128, NCH * NB * LW], bf16, tag="x",
                              name=f"x_sb_{bg}")
            for c in range(NCH):
                nc.sync.dma_start(x_sb[:, c * NB * LW:(c + 1) * NB * LW],
                                  X.ap()[bg, c])
            wm_sb = mpool.tile([128, 3 * NB * SL], bf16, tag="wm",
                               name=f"wm_sb_{bg}")
            nc.sync.dma_start(wm_sb[:], WM.ap()[bg])
            if bg == 0:
                # stream the rest of the weights in behind x/wm for bg0
                for wi in range(1, len(wt_bounds) - 1):
                    nc.sync.dma_start(
                        wt_sb[:, wt_bounds[wi] * 128:wt_bounds[wi + 1] * 128],
                        WT.ap()[:, wt_bounds[wi] * 128:wt_bounds[wi + 1] * 128])
                nc.sync.dma_start(fcw_sb[:], FCW.ap()[:])
                for j in range(5):
                    nc.sync.dma_start(
                        feat32[:, (18 + j) * BC:(19 + j) * BC],
                        E12.ap()[j * 128:(j + 1) * 128, :])
            xv = x_sb[:].rearrange("p (c b w) -> p c b w", c=NCH, b=NB)

            def pool_quad(bg, ki, h, q, ps):
                th = tpool.tile([128, NF], bf16, tag="th",
                                name=f"th_{bg}_{ki}_{h}_{q}")
                nc.scalar.activation(
                    th[:], ps[:], AF.Tanh,
                    bias=cb_sb[:, ki * 2 + h:ki * 2 + h + 1])
                for seg in range(3):
                    pr = prpool.tile([128, NF], bf16, tag="pr",
                                     name=f"pr_{bg}_{ki}_{h}_{q}_{seg}")
                    nc.vector.tensor_tensor(
                        pr[:], th[:],
                        wm_sb[:, seg * NB * SL + q * NF:
                              seg * NB * SL + (q + 1) * NF],
                        op=ALU.mult)
                    ch = ki * 6 + seg * 2 + h
                    col0 = ch * BC + bg * NB + q * 4
                    nc.vector.tensor_reduce(
                        feat32[:, col0:col0 + 4],
                        pr[:].rearrange("p (b w) -> p b w", w=SL),
                        axis=AX.X, op=ALU.add)

            def conv_group(bg, ki, h, q_outer):
                k = KS[ki]
                nsets = k * NCH
                sets = [(t, c) for t in range(k) for c in range(NCH)]
                pss = [pspool.tile([128, NF], f32, tag="ps",
                                   name=f"ps_{bg}_{ki}_{h}_{q}")
                       for q in range(NQ)]
                if q_outer:
                    # staggered: each quad's matmuls complete in turn so
                    # pooling overlaps the remaining quads' matmuls
                    for q in range(NQ):
                        for si, (t, c) in enumerate(sets):
                            s = t - k // 2
                            bi = _bidx(ki, h, t, c)
                            rhs = xv[:, c, q * 4:(q + 1) * 4,
                                     PAD + s:PAD + s + SL]
                            nc.tensor.matmul(pss[q][:],
                                             wt_sb[:, bi * 128:(bi + 1) * 128],
                                             rhs, start=(si == 0),
                                             stop=(si == nsets - 1))
                        pool_quad(bg, ki, h, q, pss[q])
                else:
                    for si, (t, c) in enumerate(sets):
                        s = t - k // 2
                        bi = _bidx(ki, h, t, c)
                        wblk = wt_sb[:, bi * 128:(bi + 1) * 128]
                        for q in range(NQ):
                            rhs = xv[:, c, q * 4:(q + 1) * 4,
                                     PAD + s:PAD + s + SL]
                            nc.tensor.matmul(pss[q][:], wblk, rhs,
                                             start=(si == 0),
                                             stop=(si == nsets - 1))
                    for q in range(NQ):
                        pool_quad(bg, ki, h, q, pss[q])

            def fc_half(half):
                # cast + FC for batch range [half*32, half*32+32)
                b0 = half * 32
                nc.vector.tensor_copy(
                    featbf[:].rearrange("p (c b) -> p c b",
                                        c=NCHK)[:, :, b0:b0 + 32],
                    feat32[:].rearrange("p (c b) -> p c b",
                                        c=NCHK)[:, :, b0:b0 + 32])
                for ch in range(NCHK):
                    nc.tensor.matmul(
                        fcps[b0:b0 + 32, :],
                        featbf[:, ch * BC + b0:ch * BC + b0 + 32],
                        fcw_sb[:, ch * VR:(ch + 1) * VR],
                        start=(ch == 0), stop=(ch == NCHK - 1))

            for gi, (ki, h) in enumerate([(ki, h) for ki in range(3)
                                          for h in range(2)]):
                final = (bg == NBG - 1) and (ki, h) == (2, 1)
                conv_group(bg, ki, h, q_outer=final)
                if bg == 2 and gi == 0:
                    # batch half 0 was fully pooled at end of bg1; its cast
                    # dep is ready by now so no PE bubble
                    fc_half(0)

        fc_half(1)
        mx = const.tile([BC, 1], f32)
        nc.vector.tensor_reduce(mx[:], fcps[:], axis=AX.X, op=ALU.max,
                                negate=True)
        esm = const.tile([BC, VR], f32)
        ssum = const.tile([BC, 1], f32)
        nc.scalar.activation(esm[:], fcps[:], AF.Exp, bias=mx[:],
                             accum_out=ssum[:])
        rin = const.tile([BC, 1], f32)
        nc.vector.reciprocal(rin[:], ssum[:])
        osb = const.tile([BC, VR], f32)
        nc.vector.tensor_scalar_mul(osb[:], esm[:], rin[:])
        nc.sync.dma_start(OUT.ap()[:], osb[:])

    nc.compile()
    return nc


_NC_CACHE = []


def _get_program():
    if not _NC_CACHE:
        _NC_CACHE.append(_build_program())
    return _NC_CACHE[0]


def _prep_inputs(W, e1, e2, pos_emb1, pos_emb2, conv_ws, conv_bs, fc_w, fc_b,
                 W_pos1, W_pos2, e1_p, e2_p):
    """Host-side data layout; returns per-core input maps."""
    # --- conv input: [B, 1024, 128] zero-padded, 3 zero cols each side ---
    Wp1 = pos_emb1[W_pos1]          # [B, S, DP]
    Wp2 = pos_emb2[W_pos2]
    Xf = np.concatenate([W, Wp1, Wp2], axis=2).transpose(0, 2, 1)  # [B,CIN,S]
    Xpad = np.zeros((B, CINP, LW), np.float32)
    Xpad[:, :CIN, PAD:PAD + S] = Xf
    Xpad = Xpad.astype(BF16).reshape(NCORE, NBG, NB, NCH, 128, LW)
    Xc = np.ascontiguousarray(Xpad.transpose(0, 1, 3, 4, 2, 5)).reshape(
        NCORE, NBG, NCH, 128, NB * LW)

    # --- segment weight masks m/cnt, replicated over 128 partitions ---
    d1 = np.minimum(e1_p, e2_p).astype(np.int64)
    d2 = np.maximum(e1_p, e2_p).astype(np.int64)
    idx = np.arange(S)[None, :]
    m1 = (idx < d1[:, None])
    m2 = (idx >= d1[:, None]) & (idx < d2[:, None])
    m3 = (idx >= d2[:, None]) & (idx < S - 1)
    wm = np.stack([m1, m2, m3], axis=1).astype(np.float32)  # [B,3,S]
    cnt = np.maximum(wm.sum(axis=2), 1.0)
    wm /= cnt[:, :, None]
    wm = wm[:, :, :SL]  # l=127 is never pooled
    wm = wm.astype(BF16).reshape(NCORE, NBG, NB, 3, SL)
    wm = np.ascontiguousarray(wm.transpose(0, 1, 3, 2, 4)).reshape(
        NCORE, NBG, 1, 3 * NB * SL)
    WMc = np.ascontiguousarray(np.broadcast_to(
        wm, (NCORE, NBG, 128, 3 * NB * SL)))

    # --- conv weights -> stationary blocks [128ci, 128co], bf16 ---
    wt = np.zeros((128, NSETS * 128), np.float32)
    for ki, k in enumerate(KS):
        cw = np.zeros((DC, CINP, k), np.float32)
        cw[:, :CIN, :] = conv_ws[ki]
        for h in range(2):
            for t in range(k):
                for c in range(NCH):
                    blk = cw[h * 128:(h + 1) * 128,
                             c * 128:(c + 1) * 128, t]  # [co, ci]
                    wt[:, _bidx(ki, h, t, c) * 128:
                       (_bidx(ki, h, t, c) + 1) * 128] = blk.T
    wt = wt.astype(BF16)

    # --- fc weights in device feature order; fc_b via constant-1 feature ---
    # f' in [0, 2304): ch = ki*6+seg*2+h, p = co_local
    #   orig col = 600 + ki*768 + (h*128+p)*3 + seg
    # f' in [2304, 2904): orig col = f' - 2304   (e1, e2)
    # f' == 2904: constant-1 -> fc_b
    fcw = np.zeros((NCHK * 128, VR), np.float32)
    fp = np.arange(F_CONV)
    ch = fp // 128
    p = fp % 128
    ki = ch // 6
    seg = (ch % 6) // 2
    h = ch % 2
    orig = 600 + ki * 768 + (h * 128 + p) * 3 + seg
    fcw[fp] = fc_w[:, orig].T
    fcw[F_CONV:F_CONV + 600] = fc_w[:, :600].T
    fcw[F_CONV + 600] = fc_b
    fcw_host = np.ascontiguousarray(
        fcw.reshape(NCHK, 128, VR).transpose(1, 0, 2)).reshape(
        128, NCHK * VR).astype(BF16)

    # --- e1/e2 + constant-1 features, fp32, per core [640, BC] ---
    e12 = np.zeros((B, 5 * 128), np.float32)
    e12[:, :300] = e1
    e12[:, 300:600] = e2
    e12[:, 600] = 1.0
    E12c = np.ascontiguousarray(
        e12.reshape(NCORE, BC, 5 * 128).transpose(0, 2, 1))

    # --- conv biases [128, 6] fp32 ---
    cb = np.zeros((128, 6), np.float32)
    for ki in range(3):
        for h in range(2):
            cb[:, ki * 2 + h] = conv_bs[ki][h * 128:(h + 1) * 128]

    in_maps = []
    for i in range(NCORE):
        in_maps.append({
            "X": Xc[i], "WM": WMc[i], "WT": wt, "FCW": fcw_host,
            "E12": E12c[i], "CB": cb,
        })
    return in_maps


def kernel(**inputs):
    f = {k: np.asarray(v) for k, v in inputs.items()}
    in_maps = _prep_inputs(
        f["W"].astype(np.float32), f["e1"].astype(np.float32),
        f["e2"].astype(np.float32), f["pos_emb1"].astype(np.float32),
        f["pos_emb2"].astype(np.float32),
        [f["conv_w3"], f["conv_w5"], f["conv_w7"]],
        [f["conv_b3"], f["conv_b5"], f["conv_b7"]],
        f["fc_w"].astype(np.float32), f["fc_b"].astype(np.float32),
        f["W_pos1"], f["W_pos2"], f["e1_p"], f["e2_p"])

    from concourse.bass_utils import run_bass_kernel_spmd
    nc = _get_program()
    try:
        res = run_bass_kernel_spmd(nc, in_maps, core_ids=list(range(NCORE)))
    except Exception:
        # transient device wedge (e.g. NRT_EXEC_UNIT_UNRECOVERABLE from a
        # prior crashed process) usually clears on retry
        res = run_bass_kernel_spmd(nc, in_maps, core_ids=list(range(NCORE)))
    out = np.concatenate([res.results[i]["OUT"] for i in range(NCORE)],
                         axis=0)
    return out.astype(np.float32)


# revision 14
# speedup vs baseline: 1.0422x; 1.0072x over previous
"""Trainium2 Bass kernel for the CNN/segment-reduce model.

Strategy (pure data-parallel over batch, 8 cores x 64 batch elems):
  host:   gather pos embeddings, transpose/concat/zero-pad the conv input to
          [ci=1024, l=128] per batch elem (3 zero cols each side for 'same'
          conv padding up to k=7), precompute segment weight-masks m/cnt,
          reorder fc_w columns to the on-device feature layout (fc_b folded
          in via a constant-1 feature), convert PE-facing data to bf16.
  device: conv = PE matmuls, contraction over 8 ci-chunks x k taps with the
          [128ci,128co] weight block stationary; rhs = shifted x window over
          4 batch elems (N=512); accumulate in PSUM [128co, 4x128l].
          ACT tanh(+bias) -> bf16 SBUF; DVE mask-multiply + segment-reduce
          -> feature tile [128, 23*64]; FC = 23 accumulating matmuls into
          PSUM [64b, 19]; fused exp/sum softmax; DMA out fp32.
"""

import numpy as np
import ml_dtypes

B, S, DW, DP, DC, VP, VR = 512, 128, 300, 50, 256, 256, 19
KS = (3, 5, 7)
CIN = 3 * DW + 2 * DP  # 1000
CINP = 1024            # padded with zero channels
NCH = CINP // 128      # 8 contraction chunks
NCORE = 8
BC = B // NCORE        # 64 batch elems per core
NB = 16                # batch elems per resident x group
NBG = BC // NB         # 4 groups
NQ = NB // 4           # 4 psum quads (4 b per N=512 matmul)
LW = S + 6             # 3 zero cols each side
PAD = 3
SL = S - 1             # conv cols actually consumed (l=127 never pooled)
NF = 4 * SL            # matmul free size (4 batch elems)
NSETS = sum(k * NCH for k in KS) * 2          # 240 weight blocks
NCHK = 23                                     # feature chunks of 128
F_CONV = 18 * 128                             # 2304 conv features
BF16 = ml_dtypes.bfloat16

# weight block index: ordered (ki-major, h, t, c)
_PREFIX = {}
_off = 0
for _ki, _k in enumerate(KS):
    for _h in range(2):
        _PREFIX[(_ki, _h)] = _off
        _off += _k * NCH


def _bidx(ki, h, t, c):
    return _PREFIX[(ki, h)] + t * NCH + c


def _build_program():
    from contextlib import ExitStack
    import concourse.tile as tile
    from concourse import bacc, mybir

    f32 = mybir.dt.float32
    bf16 = mybir.dt.bfloat16
    AF = mybir.ActivationFunctionType
    ALU = mybir.AluOpType
    AX = mybir.AxisListType

    nc = bacc.Bacc("TRN2", target_bir_lowering=False, debug=False,
                   num_devices=NCORE)

    X = nc.declare_dram_parameter("X", [NBG, NCH, 128, NB * LW], bf16,
                                  isOutput=False)
    WM = nc.declare_dram_parameter("WM", [NBG, 128, 3 * NB * SL], bf16,
                                   isOutput=False)
    WT = nc.declare_dram_parameter("WT", [128, NSETS * 128], bf16,
                                   isOutput=False)
    FCW = nc.declare_dram_parameter("FCW", [128, NCHK * VR], bf16,
                                    isOutput=False)
    E12 = nc.declare_dram_parameter("E12", [5 * 128, BC], f32, isOutput=False)
    CB = nc.declare_dram_parameter("CB", [128, 6], f32, isOutput=False)
    OUT = nc.declare_dram_parameter("OUT", [BC, VR], f32, isOutput=True)

    with tile.TileContext(nc) as tc, ExitStack() as ctx:
        const = ctx.enter_context(tc.tile_pool(name="const", bufs=1))
        xpool = ctx.enter_context(tc.tile_pool(name="xp", bufs=2))
        mpool = ctx.enter_context(tc.tile_pool(name="mp", bufs=2))
        tpool = ctx.enter_context(tc.tile_pool(name="tp", bufs=4))
        prpool = ctx.enter_context(tc.tile_pool(name="prp", bufs=4))
        pspool = ctx.enter_context(tc.tile_pool(name="psp", bufs=8,
                                                space="PSUM"))

        # PE warm-up: dummy matmuls with no DMA deps so the HAM clock-gate
        # flips to 8/8 while the first weight/x DMAs are in flight.
        warm = const.tile([128, NF], bf16)
        nc.any.memset(warm[:], 0.0)
        wps = pspool.tile([128, NF], f32, tag="ps")
        for _ in range(10):
            nc.tensor.matmul(wps[:], warm[:, 0:128], warm[:],
                             start=True, stop=True)

        wt_sb = const.tile([128, NSETS * 128], bf16)
        # one DMA per (ki,h) group so each group unblocks as its slice lands
        wt_bounds = sorted(_PREFIX.values()) + [NSETS]
        nc.sync.dma_start(wt_sb[:, 0:wt_bounds[1] * 128],
                          WT.ap()[:, 0:wt_bounds[1] * 128])
        fcw_sb = const.tile([128, NCHK * VR], bf16)
        cb_sb = const.tile([128, 6], f32)
        nc.sync.dma_start(cb_sb[:], CB.ap()[:])
        feat32 = const.tile([128, NCHK * BC], f32)
        featbf = const.tile([128, NCHK * BC], bf16)

        fcps = pspool.tile([BC, VR], f32, tag="ps")
        for bg in range(NBG):
            x_sb = xpool.tile([128, NCH * NB * LW], bf16, tag="x",
                              name=f"x_sb_{bg}")
            for c in range(NCH):
                nc.sync.dma_start(x_sb[:, c * NB * LW:(c + 1) * NB * LW],
                                  X.ap()[bg, c])
            wm_sb = mpool.tile([128, 3 * NB * SL], bf16, tag="wm",
                               name=f"wm_sb_{bg}")
            nc.sync.dma_start(wm_sb[:], WM.ap()[bg])
            if bg == 0:
                # stream the rest of the weights in behind x/wm for bg0
                for wi in range(1, len(wt_bounds) - 1):
                    nc.sync.dma_start(
                        wt_sb[:, wt_bounds[wi] * 128:wt_bounds[wi + 1] * 128],
                        WT.ap()[:, wt_bounds[wi] * 128:wt_bounds[wi + 1] * 128])
                nc.sync.dma_start(fcw_sb[:], FCW.ap()[:])
                for j in range(5):
                    nc.sync.dma_start(
                        feat32[:, (18 + j) * BC:(19 + j) * BC],
                        E12.ap()[j * 128:(j + 1) * 128, :])
            xv = x_sb[:].rearrange("p (c b w) -> p c b w", c=NCH, b=NB)

            def pool_quad(bg, ki, h, q, ps):
                th = tpool.tile([128, NF], bf16, tag="th",
                                name=f"th_{bg}_{ki}_{h}_{q}")
                nc.scalar.activation(
                    th[:], ps[:], AF.Tanh,
                    bias=cb_sb[:, ki * 2 + h:ki * 2 + h + 1])
                for seg in range(3):
                    pr = prpool.tile([128, NF], bf16, tag="pr",
                                     name=f"pr_{bg}_{ki}_{h}_{q}_{seg}")
                    nc.vector.tensor_tensor(
                        pr[:], th[:],
                        wm_sb[:, seg * NB * SL + q * NF:
                              seg * NB * SL + (q + 1) * NF],
                        op=ALU.mult)
                    ch = ki * 6 + seg * 2 + h
                    col0 = ch * BC + bg * NB + q * 4
                    nc.vector.tensor_reduce(
                        feat32[:, col0:col0 + 4],
                        pr[:].rearrange("p (b w) -> p b w", w=SL),
                        axis=AX.X, op=ALU.add)

            def conv_group(bg, ki, h, q_outer):
                k = KS[ki]
                nsets = k * NCH
                # c-major: early sets touch only early x chunks, so the
                # cold-start conv isn't gated on the full x DMA
                sets = [(t, c) for c in range(NCH) for t in range(k)]
                pss = [pspool.tile([128, NF], f32, tag="ps",
                                   name=f"ps_{bg}_{ki}_{h}_{q}")
                       for q in range(NQ)]
                if q_outer:
                    # staggered: each quad's matmuls complete in turn so
                    # pooling overlaps the remaining quads' matmuls
                    for q in range(NQ):
                        for si, (t, c) in enumerate(sets):
                            s = t - k // 2
                            bi = _bidx(ki, h, t, c)
                            rhs = xv[:, c, q * 4:(q + 1) * 4,
                                     PAD + s:PAD + s + SL]
                            nc.tensor.matmul(pss[q][:],
                                             wt_sb[:, bi * 128:(bi + 1) * 128],
                                             rhs, start=(si == 0),
                                             stop=(si == nsets - 1))
                        pool_quad(bg, ki, h, q, pss[q])
                else:
                    for si, (t, c) in enumerate(sets):
                        s = t - k // 2
                        bi = _bidx(ki, h, t, c)
                        wblk = wt_sb[:, bi * 128:(bi + 1) * 128]
                        for q in range(NQ):
                            rhs = xv[:, c, q * 4:(q + 1) * 4,
                                     PAD + s:PAD + s + SL]
                            nc.tensor.matmul(pss[q][:], wblk, rhs,
                                             start=(si == 0),
                                             stop=(si == nsets - 1))
                    for q in range(NQ):
                        pool_quad(bg, ki, h, q, pss[q])

            def cast_cols(b0, nb):
                nc.vector.tensor_copy(
                    featbf[:].rearrange("p (c b) -> p c b",
                                        c=NCHK)[:, :, b0:b0 + nb],
                    feat32[:].rearrange("p (c b) -> p c b",
                                        c=NCHK)[:, :, b0:b0 + nb])

            def fc_half(half):
                b0 = half * 32
                for ch in range(NCHK):
                    nc.tensor.matmul(
                        fcps[b0:b0 + 32, :],
                        featbf[:, ch * BC + b0:ch * BC + b0 + 32],
                        fcw_sb[:, ch * VR:(ch + 1) * VR],
                        start=(ch == 0), stop=(ch == NCHK - 1))

            for gi, (ki, h) in enumerate([(ki, h) for ki in range(3)
                                          for h in range(2)]):
                final = (bg == NBG - 1) and (ki, h) == (2, 1)
                conv_group(bg, ki, h, q_outer=final)
                if gi == 0 and bg == 2:
                    # batch half 0 fully pooled at end of bg1; cast dep is
                    # ready by now so no PE bubble
                    cast_cols(0, 32)
                    fc_half(0)
                if gi == 0 and bg == 3:
                    cast_cols(32, 16)  # b 32..47 pooled at end of bg2

        cast_cols(48, 16)
        fc_half(1)
        mx = const.tile([BC, 1], f32)
        nc.vector.tensor_reduce(mx[:], fcps[:], axis=AX.X, op=ALU.max,
                                negate=True)
        esm = const.tile([BC, VR], f32)
        ssum = const.tile([BC, 1], f32)
        nc.scalar.activation(esm[:], fcps[:], AF.Exp, bias=mx[:],
                             accum_out=ssum[:])
        rin = const.tile([BC, 1], f32)
        nc.vector.reciprocal(rin[:], ssum[:])
        osb = const.tile([BC, VR], f32)
        nc.vector.tensor_scalar_mul(osb[:], esm[:], rin[:])
        nc.sync.dma_start(OUT.ap()[:], osb[:])

    nc.compile()
    return nc


_NC_CACHE = []


def _get_program():
    if not _NC_CACHE:
        _NC_CACHE.append(_build_program())
    return _NC_CACHE[0]


def _prep_inputs(W, e1, e2, pos_emb1, pos_emb2, conv_ws, conv_bs, fc_w, fc_b,
                 W_pos1, W_pos2, e1_p, e2_p):
    """Host-side data layout; returns per-core input maps."""
    # --- conv input: [B, 1024, 128] zero-padded, 3 zero cols each side ---
    Wp1 = pos_emb1[W_pos1]          # [B, S, DP]
    Wp2 = pos_emb2[W_pos2]
    Xf = np.concatenate([W, Wp1, Wp2], axis=2).transpose(0, 2, 1)  # [B,CIN,S]
    Xpad = np.zeros((B, CINP, LW), np.float32)
    Xpad[:, :CIN, PAD:PAD + S] = Xf
    Xpad = Xpad.astype(BF16).reshape(NCORE, NBG, NB, NCH, 128, LW)
    Xc = np.ascontiguousarray(Xpad.transpose(0, 1, 3, 4, 2, 5)).reshape(
        NCORE, NBG, NCH, 128, NB * LW)

    # --- segment weight masks m/cnt, replicated over 128 partitions ---
    d1 = np.minimum(e1_p, e2_p).astype(np.int64)
    d2 = np.maximum(e1_p, e2_p).astype(np.int64)
    idx = np.arange(S)[None, :]
    m1 = (idx < d1[:, None])
    m2 = (idx >= d1[:, None]) & (idx < d2[:, None])
    m3 = (idx >= d2[:, None]) & (idx < S - 1)
    wm = np.stack([m1, m2, m3], axis=1).astype(np.float32)  # [B,3,S]
    cnt = np.maximum(wm.sum(axis=2), 1.0)
    wm /= cnt[:, :, None]
    wm = wm[:, :, :SL]  # l=127 is never pooled
    wm = wm.astype(BF16).reshape(NCORE, NBG, NB, 3, SL)
    wm = np.ascontiguousarray(wm.transpose(0, 1, 3, 2, 4)).reshape(
        NCORE, NBG, 1, 3 * NB * SL)
    WMc = np.ascontiguousarray(np.broadcast_to(
        wm, (NCORE, NBG, 128, 3 * NB * SL)))

    # --- conv weights -> stationary blocks [128ci, 128co], bf16 ---
    wt = np.zeros((128, NSETS * 128), np.float32)
    for ki, k in enumerate(KS):
        cw = np.zeros((DC, CINP, k), np.float32)
        cw[:, :CIN, :] = conv_ws[ki]
        for h in range(2):
            for t in range(k):
                for c in range(NCH):
                    blk = cw[h * 128:(h + 1) * 128,
                             c * 128:(c + 1) * 128, t]  # [co, ci]
                    wt[:, _bidx(ki, h, t, c) * 128:
                       (_bidx(ki, h, t, c) + 1) * 128] = blk.T
    wt = wt.astype(BF16)

    # --- fc weights in device feature order; fc_b via constant-1 feature ---
    # f' in [0, 2304): ch = ki*6+seg*2+h, p = co_local
    #   orig col = 600 + ki*768 + (h*128+p)*3 + seg
    # f' in [2304, 2904): orig col = f' - 2304   (e1, e2)
    # f' == 2904: constant-1 -> fc_b
    fcw = np.zeros((NCHK * 128, VR), np.float32)
    fp = np.arange(F_CONV)
    ch = fp // 128
    p = fp % 128
    ki = ch // 6
    seg = (ch % 6) // 2
    h = ch % 2
    orig = 600 + ki * 768 + (h * 128 + p) * 3 + seg
    fcw[fp] = fc_w[:, orig].T
    fcw[F_CONV:F_CONV + 600] = fc_w[:, :600].T
    fcw[F_CONV + 600] = fc_b
    fcw_host = np.ascontiguousarray(
        fcw.reshape(NCHK, 128, VR).transpose(1, 0, 2)).reshape(
        128, NCHK * VR).astype(BF16)

    # --- e1/e2 + constant-1 features, fp32, per core [640, BC] ---
    e12 = np.zeros((B, 5 * 128), np.float32)
    e12[:, :300] = e1
    e12[:, 300:600] = e2
    e12[:, 600] = 1.0
    E12c = np.ascontiguousarray(
        e12.reshape(NCORE, BC, 5 * 128).transpose(0, 2, 1))

    # --- conv biases [128, 6] fp32 ---
    cb = np.zeros((128, 6), np.float32)
    for ki in range(3):
        for h in range(2):
            cb[:, ki * 2 + h] = conv_bs[ki][h * 128:(h + 1) * 128]

    in_maps = []
    for i in range(NCORE):
        in_maps.append({
            "X": Xc[i], "WM": WMc[i], "WT": wt, "FCW": fcw_host,
            "E12": E12c[i], "CB": cb,
        })
    return in_maps


def kernel(**inputs):
    f = {k: np.asarray(v) for k, v in inputs.items()}
    in_maps = _prep_inputs(
        f["W"].astype(np.float32), f["e1"].astype(np.float32),
        f["e2"].astype(np.float32), f["pos_emb1"].astype(np.float32),
        f["pos_emb2"].astype(np.float32),
        [f["conv_w3"], f["conv_w5"], f["conv_w7"]],
        [f["conv_b3"], f["conv_b5"], f["conv_b7"]],
        f["fc_w"].astype(np.float32), f["fc_b"].astype(np.float32),
        f["W_pos1"], f["W_pos2"], f["e1_p"], f["e2_p"])

    from concourse.bass_utils import run_bass_kernel_spmd
    nc = _get_program()
    try:
        res = run_bass_kernel_spmd(nc, in_maps, core_ids=list(range(NCORE)))
    except Exception:
        # transient device wedge (e.g. NRT_EXEC_UNIT_UNRECOVERABLE from a
        # prior crashed process) usually clears on retry
        res = run_bass_kernel_spmd(nc, in_maps, core_ids=list(range(NCORE)))
    out = np.concatenate([res.results[i]["OUT"] for i in range(NCORE)],
                         axis=0)
    return out.astype(np.float32)


# revision 21
# speedup vs baseline: 1.0459x; 1.0036x over previous
"""Trainium2 Bass kernel for the CNN/segment-reduce model.

Strategy (pure data-parallel over batch, 8 cores x 64 batch elems):
  host:   gather pos embeddings, transpose/concat/zero-pad the conv input to
          [ci=1024, l=128] per batch elem (3 zero cols each side for 'same'
          conv padding up to k=7), precompute segment weight-masks m/cnt,
          reorder fc_w columns to the on-device feature layout (fc_b folded
          in via a constant-1 feature), convert PE-facing data to bf16.
  device: conv = PE matmuls, contraction over 8 ci-chunks x k taps with the
          [128ci,128co] weight block stationary; rhs = shifted x window over
          4 batch elems (N=512); accumulate in PSUM [128co, 4x128l].
          ACT tanh(+bias) -> bf16 SBUF; DVE mask-multiply + segment-reduce
          -> feature tile [128, 23*64]; FC = 23 accumulating matmuls into
          PSUM [64b, 19]; fused exp/sum softmax; DMA out fp32.
"""

import numpy as np
import ml_dtypes

B, S, DW, DP, DC, VP, VR = 512, 128, 300, 50, 256, 256, 19
KS = (3, 5, 7)
CIN = 3 * DW + 2 * DP  # 1000
CINP = 1024            # padded with zero channels
NCH = CINP // 128      # 8 contraction chunks
NCORE = 8
BC = B // NCORE        # 64 batch elems per core
NB = 16                # batch elems per resident x group
NBG = BC // NB         # 4 groups
NQ = NB // 4           # 4 psum quads (4 b per N=512 matmul)
LW = S + 6             # 3 zero cols each side
PAD = 3
SL = S - 1             # conv cols actually consumed (l=127 never pooled)
NF = 4 * SL            # matmul free size (4 batch elems)
NSETS = sum(k * NCH for k in KS) * 2          # 240 weight blocks
NCHK = 23                                     # feature chunks of 128
F_CONV = 18 * 128                             # 2304 conv features
BF16 = ml_dtypes.bfloat16

# weight block index: ordered (ki-major, h, t, c)
_PREFIX = {}
_off = 0
for _ki, _k in enumerate(KS):
    for _h in range(2):
        _PREFIX[(_ki, _h)] = _off
        _off += _k * NCH


def _bidx(ki, h, t, c):
    return _PREFIX[(ki, h)] + t * NCH + c


def _build_program():
    from contextlib import ExitStack
    import concourse.tile as tile
    from concourse import bacc, mybir

    f32 = mybir.dt.float32
    bf16 = mybir.dt.bfloat16
    AF = mybir.ActivationFunctionType
    ALU = mybir.AluOpType
    AX = mybir.AxisListType

    nc = bacc.Bacc("TRN2", target_bir_lowering=False, debug=False,
                   num_devices=NCORE)

    X = nc.declare_dram_parameter("X", [NBG, NCH, 128, NB * LW], bf16,
                                  isOutput=False)
    WM = nc.declare_dram_parameter("WM", [NBG, 128, 3 * NB * SL], bf16,
                                   isOutput=False)
    WT = nc.declare_dram_parameter("WT", [128, NSETS * 128], bf16,
                                   isOutput=False)
    FCW = nc.declare_dram_parameter("FCW", [128, NCHK * VR], bf16,
                                    isOutput=False)
    E12 = nc.declare_dram_parameter("E12", [5 * 128, BC], f32, isOutput=False)
    CB = nc.declare_dram_parameter("CB", [128, 6], f32, isOutput=False)
    OUT = nc.declare_dram_parameter("OUT", [BC, VR], f32, isOutput=True)

    with tile.TileContext(nc) as tc, ExitStack() as ctx:
        const = ctx.enter_context(tc.tile_pool(name="const", bufs=1))
        xpool = ctx.enter_context(tc.tile_pool(name="xp", bufs=2))
        mpool = ctx.enter_context(tc.tile_pool(name="mp", bufs=2))
        tpool = ctx.enter_context(tc.tile_pool(name="tp", bufs=4))
        prpool = ctx.enter_context(tc.tile_pool(name="prp", bufs=4))
        pspool = ctx.enter_context(tc.tile_pool(name="psp", bufs=8,
                                                space="PSUM"))

        # PE warm-up: dummy matmuls with no DMA deps so the HAM clock-gate
        # flips to 8/8 while the first weight/x DMAs are in flight.
        warm = const.tile([128, NF], bf16)
        nc.any.memset(warm[:], 0.0)
        wps = pspool.tile([128, NF], f32, tag="ps")
        for _ in range(10):
            nc.tensor.matmul(wps[:], warm[:, 0:128], warm[:],
                             start=True, stop=True)

        wt_sb = const.tile([128, NSETS * 128], bf16)
        wt_bounds = sorted(_PREFIX.values()) + [NSETS]
        fcw_sb = const.tile([128, NCHK * VR], bf16)
        cb_sb = const.tile([128, 6], f32)
        feat32 = const.tile([128, NCHK * BC], f32)
        featbf = const.tile([128, NCHK * BC], bf16)

        fcps = pspool.tile([BC, VR], f32, tag="ps")
        for bg in range(NBG):
            x_sb = xpool.tile([128, NCH * NB * LW], bf16, tag="x",
                              name=f"x_sb_{bg}")
            if bg == 0:
                # x chunk 0 + first weight group race in first, in parallel
                nc.sync.dma_start(x_sb[:, 0:NB * LW], X.ap()[0, 0])
                nc.sync.dma_start(wt_sb[:, 0:wt_bounds[1] * 128],
                                  WT.ap()[:, 0:wt_bounds[1] * 128])
                nc.sync.dma_start(cb_sb[:], CB.ap()[:])
            for c in range(0 if bg else 1, NCH):
                nc.sync.dma_start(x_sb[:, c * NB * LW:(c + 1) * NB * LW],
                                  X.ap()[bg, c])
            wm_sb = mpool.tile([128, 3 * NB * SL], bf16, tag="wm",
                               name=f"wm_sb_{bg}")
            nc.sync.dma_start(wm_sb[:], WM.ap()[bg])
            if bg == 0:
                # stream the rest of the weights in behind x/wm for bg0
                for wi in range(1, len(wt_bounds) - 1):
                    nc.sync.dma_start(
                        wt_sb[:, wt_bounds[wi] * 128:wt_bounds[wi + 1] * 128],
                        WT.ap()[:, wt_bounds[wi] * 128:wt_bounds[wi + 1] * 128])
                nc.sync.dma_start(fcw_sb[:], FCW.ap()[:])
                for j in range(5):
                    nc.sync.dma_start(
                        feat32[:, (18 + j) * BC:(19 + j) * BC],
                        E12.ap()[j * 128:(j + 1) * 128, :])
            xv = x_sb[:].rearrange("p (c b w) -> p c b w", c=NCH, b=NB)

            def pool_quad(bg, ki, h, q, ps, fused=False):
                th = tpool.tile([128, NF], bf16, tag="th",
                                name=f"th_{bg}_{ki}_{h}_{q}")
                nc.scalar.activation(
                    th[:], ps[:], AF.Tanh,
                    bias=cb_sb[:, ki * 2 + h:ki * 2 + h + 1])
                for seg in range(3):
                    ch = ki * 6 + seg * 2 + h
                    col0 = ch * BC + bg * NB + q * 4
                    wmq = wm_sb[:, seg * NB * SL + q * NF:
                                seg * NB * SL + (q + 1) * NF]
                    pr = prpool.tile([128, NF], bf16, tag="pr",
                                     name=f"pr_{bg}_{ki}_{h}_{q}_{seg}")
                    nc.vector.tensor_tensor(pr[:], th[:], wmq,
                                            op=ALU.mult)
                    nc.vector.tensor_reduce(
                        feat32[:, col0:col0 + 4],
                        pr[:].rearrange("p (b w) -> p b w", w=SL),
                        axis=AX.X, op=ALU.add)

            def conv_group(bg, ki, h, q_outer):
                k = KS[ki]
                nsets = k * NCH
                # c-major: early sets touch only early x chunks, so the
                # cold-start conv isn't gated on the full x DMA
                sets = [(t, c) for c in range(NCH) for t in range(k)]
                pss = [pspool.tile([128, NF], f32, tag="ps",
                                   name=f"ps_{bg}_{ki}_{h}_{q}")
                       for q in range(NQ)]
                if q_outer:
                    # staggered: each quad's matmuls complete in turn so
                    # pooling overlaps the remaining quads' matmuls
                    for q in range(NQ):
                        for si, (t, c) in enumerate(sets):
                            s = t - k // 2
                            bi = _bidx(ki, h, t, c)
                            rhs = xv[:, c, q * 4:(q + 1) * 4,
                                     PAD + s:PAD + s + SL]
                            nc.tensor.matmul(pss[q][:],
                                             wt_sb[:, bi * 128:(bi + 1) * 128],
                                             rhs, start=(si == 0),
                                             stop=(si == nsets - 1))
                        pool_quad(bg, ki, h, q, pss[q])
                else:
                    for si, (t, c) in enumerate(sets):
                        s = t - k // 2
                        bi = _bidx(ki, h, t, c)
                        wblk = wt_sb[:, bi * 128:(bi + 1) * 128]
                        for q in range(NQ):
                            rhs = xv[:, c, q * 4:(q + 1) * 4,
                                     PAD + s:PAD + s + SL]
                            nc.tensor.matmul(pss[q][:], wblk, rhs,
                                             start=(si == 0),
                                             stop=(si == nsets - 1))
                    for q in range(NQ):
                        pool_quad(bg, ki, h, q, pss[q])

            def cast_cols(b0, nb):
                nc.vector.tensor_copy(
                    featbf[:].rearrange("p (c b) -> p c b",
                                        c=NCHK)[:, :, b0:b0 + nb],
                    feat32[:].rearrange("p (c b) -> p c b",
                                        c=NCHK)[:, :, b0:b0 + nb])

            def fc_half(half):
                b0 = half * 32
                for ch in range(NCHK):
                    nc.tensor.matmul(
                        fcps[b0:b0 + 32, :],
                        featbf[:, ch * BC + b0:ch * BC + b0 + 32],
                        fcw_sb[:, ch * VR:(ch + 1) * VR],
                        start=(ch == 0), stop=(ch == NCHK - 1))

            for gi, (ki, h) in enumerate([(ki, h) for ki in range(3)
                                          for h in range(2)]):
                final = (bg == NBG - 1) and (ki, h) == (2, 1)
                conv_group(bg, ki, h, q_outer=final)
                if gi == 0 and bg == 2:
                    # batch half 0 fully pooled at end of bg1; cast dep is
                    # ready by now so no PE bubble
                    cast_cols(0, 32)
                    fc_half(0)
                if gi == 0 and bg == 3:
                    cast_cols(32, 16)  # b 32..47 pooled at end of bg2

        cast_cols(48, 16)
        fc_half(1)
        mx = const.tile([BC, 1], f32)
        nc.vector.tensor_reduce(mx[:], fcps[:], axis=AX.X, op=ALU.max,
                                negate=True)
        esm = const.tile([BC, VR], f32)
        ssum = const.tile([BC, 1], f32)
        nc.scalar.activation(esm[:], fcps[:], AF.Exp, bias=mx[:],
                             accum_out=ssum[:])
        rin = const.tile([BC, 1], f32)
        nc.vector.reciprocal(rin[:], ssum[:])
        osb = const.tile([BC, VR], f32)
        nc.vector.tensor_scalar_mul(osb[:], esm[:], rin[:])
        nc.sync.dma_start(OUT.ap()[:], osb[:])

    nc.compile()
    return nc


_NC_CACHE = []


def _get_program():
    if not _NC_CACHE:
        _NC_CACHE.append(_build_program())
    return _NC_CACHE[0]


def _prep_inputs(W, e1, e2, pos_emb1, pos_emb2, conv_ws, conv_bs, fc_w, fc_b,
                 W_pos1, W_pos2, e1_p, e2_p):
    """Host-side data layout; returns per-core input maps."""
    # --- conv input: [B, 1024, 128] zero-padded, 3 zero cols each side ---
    Wp1 = pos_emb1[W_pos1]          # [B, S, DP]
    Wp2 = pos_emb2[W_pos2]
    Xf = np.concatenate([W, Wp1, Wp2], axis=2).transpose(0, 2, 1)  # [B,CIN,S]
    Xpad = np.zeros((B, CINP, LW), np.float32)
    Xpad[:, :CIN, PAD:PAD + S] = Xf
    Xpad = Xpad.astype(BF16).reshape(NCORE, NBG, NB, NCH, 128, LW)
    Xc = np.ascontiguousarray(Xpad.transpose(0, 1, 3, 4, 2, 5)).reshape(
        NCORE, NBG, NCH, 128, NB * LW)

    # --- segment weight masks m/cnt, replicated over 128 partitions ---
    d1 = np.minimum(e1_p, e2_p).astype(np.int64)
    d2 = np.maximum(e1_p, e2_p).astype(np.int64)
    idx = np.arange(S)[None, :]
    m1 = (idx < d1[:, None])
    m2 = (idx >= d1[:, None]) & (idx < d2[:, None])
    m3 = (idx >= d2[:, None]) & (idx < S - 1)
    wm = np.stack([m1, m2, m3], axis=1).astype(np.float32)  # [B,3,S]
    cnt = np.maximum(wm.sum(axis=2), 1.0)
    wm /= cnt[:, :, None]
    wm = wm[:, :, :SL]  # l=127 is never pooled
    wm = wm.astype(BF16).reshape(NCORE, NBG, NB, 3, SL)
    wm = np.ascontiguousarray(wm.transpose(0, 1, 3, 2, 4)).reshape(
        NCORE, NBG, 1, 3 * NB * SL)
    WMc = np.ascontiguousarray(np.broadcast_to(
        wm, (NCORE, NBG, 128, 3 * NB * SL)))

    # --- conv weights -> stationary blocks [128ci, 128co], bf16 ---
    wt = np.zeros((128, NSETS * 128), np.float32)
    for ki, k in enumerate(KS):
        cw = np.zeros((DC, CINP, k), np.float32)
        cw[:, :CIN, :] = conv_ws[ki]
        for h in range(2):
            for t in range(k):
                for c in range(NCH):
                    blk = cw[h * 128:(h + 1) * 128,
                             c * 128:(c + 1) * 128, t]  # [co, ci]
                    wt[:, _bidx(ki, h, t, c) * 128:
                       (_bidx(ki, h, t, c) + 1) * 128] = blk.T
    wt = wt.astype(BF16)

    # --- fc weights in device feature order; fc_b via constant-1 feature ---
    # f' in [0, 2304): ch = ki*6+seg*2+h, p = co_local
    #   orig col = 600 + ki*768 + (h*128+p)*3 + seg
    # f' in [2304, 2904): orig col = f' - 2304   (e1, e2)
    # f' == 2904: constant-1 -> fc_b
    fcw = np.zeros((NCHK * 128, VR), np.float32)
    fp = np.arange(F_CONV)
    ch = fp // 128
    p = fp % 128
    ki = ch // 6
    seg = (ch % 6) // 2
    h = ch % 2
    orig = 600 + ki * 768 + (h * 128 + p) * 3 + seg
    fcw[fp] = fc_w[:, orig].T
    fcw[F_CONV:F_CONV + 600] = fc_w[:, :600].T
    fcw[F_CONV + 600] = fc_b
    fcw_host = np.ascontiguousarray(
        fcw.reshape(NCHK, 128, VR).transpose(1, 0, 2)).reshape(
        128, NCHK * VR).astype(BF16)

    # --- e1/e2 + constant-1 features, fp32, per core [640, BC] ---
    e12 = np.zeros((B, 5 * 128), np.float32)
    e12[:, :300] = e1
    e12[:, 300:600] = e2
    e12[:, 600] = 1.0
    E12c = np.ascontiguousarray(
        e12.reshape(NCORE, BC, 5 * 128).transpose(0, 2, 1))

    # --- conv biases [128, 6] fp32 ---
    cb = np.zeros((128, 6), np.float32)
    for ki in range(3):
        for h in range(2):
            cb[:, ki * 2 + h] = conv_bs[ki][h * 128:(h + 1) * 128]

    in_maps = []
    for i in range(NCORE):
        in_maps.append({
            "X": Xc[i], "WM": WMc[i], "WT": wt, "FCW": fcw_host,
            "E12": E12c[i], "CB": cb,
        })
    return in_maps


def kernel(**inputs):
    f = {k: np.asarray(v) for k, v in inputs.items()}
    in_maps = _prep_inputs(
        f["W"].astype(np.float32), f["e1"].astype(np.float32),
        f["e2"].astype(np.float32), f["pos_emb1"].astype(np.float32),
        f["pos_emb2"].astype(np.float32),
        [f["conv_w3"], f["conv_w5"], f["conv_w7"]],
        [f["conv_b3"], f["conv_b5"], f["conv_b7"]],
        f["fc_w"].astype(np.float32), f["fc_b"].astype(np.float32),
        f["W_pos1"], f["W_pos2"], f["e1_p"], f["e2_p"])

    from concourse.bass_utils import run_bass_kernel_spmd
    nc = _get_program()
    try:
        res = run_bass_kernel_spmd(nc, in_maps, core_ids=list(range(NCORE)))
    except Exception:
        # device wedge (e.g. NRT_EXEC_UNIT_UNRECOVERABLE left by a prior
        # crashed process): reset the runtime, then retry once
        try:
            import ctypes
            import jax
            jax.devices()
            ctypes.CDLL("/opt/axon/libaxon_pjrt.so").axon_reset()
        except Exception:
            pass
        res = run_bass_kernel_spmd(nc, in_maps, core_ids=list(range(NCORE)))
    out = np.concatenate([res.results[i]["OUT"] for i in range(NCORE)],
                         axis=0)
    return out.astype(np.float32)


# revision 26
# speedup vs baseline: 1.0481x; 1.0021x over previous
"""Trainium2 Bass kernel for the CNN/segment-reduce model.

Strategy (pure data-parallel over batch, 8 cores x 64 batch elems):
  host:   gather pos embeddings, transpose/concat/zero-pad the conv input to
          [ci=1024, l=128] per batch elem (3 zero cols each side for 'same'
          conv padding up to k=7), precompute segment weight-masks m/cnt,
          reorder fc_w columns to the on-device feature layout (fc_b folded
          in via a constant-1 feature), convert PE-facing data to bf16.
  device: conv = PE matmuls, contraction over 8 ci-chunks x k taps with the
          [128ci,128co] weight block stationary; rhs = shifted x window over
          4 batch elems (N=512); accumulate in PSUM [128co, 4x128l].
          ACT tanh(+bias) -> bf16 SBUF; DVE mask-multiply + segment-reduce
          -> feature tile [128, 23*64]; FC = 23 accumulating matmuls into
          PSUM [64b, 19]; fused exp/sum softmax; DMA out fp32.
"""

import numpy as np
import ml_dtypes

B, S, DW, DP, DC, VP, VR = 512, 128, 300, 50, 256, 256, 19
KS = (3, 5, 7)
CIN = 3 * DW + 2 * DP  # 1000
CINP = 1024            # padded with zero channels
NCH = CINP // 128      # 8 contraction chunks
NCORE = 8
BC = B // NCORE        # 64 batch elems per core
NB = 16                # batch elems per resident x group
NBG = BC // NB         # 4 groups
NQ = NB // 4           # 4 psum quads (4 b per N=512 matmul)
LW = S + 6             # 3 zero cols each side
PAD = 3
SL = S - 1             # conv cols actually consumed (l=127 never pooled)
NF = 4 * SL            # matmul free size (4 batch elems)
NSETS = sum(k * NCH for k in KS) * 2          # 240 weight blocks
NCHK = 23                                     # feature chunks of 128
F_CONV = 18 * 128                             # 2304 conv features
BF16 = ml_dtypes.bfloat16

# weight block index: ordered (ki-major, h, t, c)
_PREFIX = {}
_off = 0
for _ki, _k in enumerate(KS):
    for _h in range(2):
        _PREFIX[(_ki, _h)] = _off
        _off += _k * NCH


def _bidx(ki, h, t, c):
    # c-major within a group: matches the device's set consumption order so
    # a partial weight DMA unblocks the first matmuls immediately
    return _PREFIX[(ki, h)] + c * KS[ki] + t


def _build_program():
    from contextlib import ExitStack
    import concourse.tile as tile
    from concourse import bacc, mybir

    f32 = mybir.dt.float32
    bf16 = mybir.dt.bfloat16
    AF = mybir.ActivationFunctionType
    ALU = mybir.AluOpType
    AX = mybir.AxisListType

    nc = bacc.Bacc("TRN2", target_bir_lowering=False, debug=False,
                   num_devices=NCORE)

    X = nc.declare_dram_parameter("X", [NBG, NCH, 128, NB * LW], bf16,
                                  isOutput=False)
    WM = nc.declare_dram_parameter("WM", [NBG, 128, 3 * NB * SL], bf16,
                                   isOutput=False)
    WT = nc.declare_dram_parameter("WT", [128, NSETS * 128], bf16,
                                   isOutput=False)
    FCW = nc.declare_dram_parameter("FCW", [128, NCHK * VR], bf16,
                                    isOutput=False)
    E12 = nc.declare_dram_parameter("E12", [5 * 128, BC], f32, isOutput=False)
    CB = nc.declare_dram_parameter("CB", [128, 6], f32, isOutput=False)
    OUT = nc.declare_dram_parameter("OUT", [BC, VR], f32, isOutput=True)

    with tile.TileContext(nc) as tc, ExitStack() as ctx:
        const = ctx.enter_context(tc.tile_pool(name="const", bufs=1))
        xpool = ctx.enter_context(tc.tile_pool(name="xp", bufs=2))
        mpool = ctx.enter_context(tc.tile_pool(name="mp", bufs=2))
        tpool = ctx.enter_context(tc.tile_pool(name="tp", bufs=4))
        prpool = ctx.enter_context(tc.tile_pool(name="prp", bufs=4))
        pspool = ctx.enter_context(tc.tile_pool(name="psp", bufs=8,
                                                space="PSUM"))

        # PE warm-up: dummy matmuls with no DMA deps so the HAM clock-gate
        # flips to 8/8 while the first weight/x DMAs are in flight.
        warm = const.tile([128, NF], bf16)
        nc.any.memset(warm[:], 0.0)
        wps = pspool.tile([128, NF], f32, tag="ps")
        for _ in range(12):
            nc.tensor.matmul(wps[:], warm[:, 0:128], warm[:],
                             start=True, stop=True)

        wt_sb = const.tile([128, NSETS * 128], bf16)
        wt_bounds = sorted(_PREFIX.values()) + [NSETS]
        fcw_sb = const.tile([128, NCHK * VR], bf16)
        cb_sb = const.tile([128, 6], f32)
        feat32 = const.tile([128, NCHK * BC], f32)
        featbf = const.tile([128, NCHK * BC], bf16)

        fcps = pspool.tile([BC, VR], f32, tag="ps")
        for bg in range(NBG):
            x_sb = xpool.tile([128, NCH * NB * LW], bf16, tag="x",
                              name=f"x_sb_{bg}")
            if bg == 0:
                # x chunk 0 + the first few weight blocks race in first;
                # blocks are in consumption order, so 6 blocks cover the
                # first two x chunks' worth of matmuls
                nc.sync.dma_start(x_sb[:, 0:NB * LW], X.ap()[0, 0])
                nc.sync.dma_start(wt_sb[:, 0:6 * 128], WT.ap()[:, 0:6 * 128])
                nc.sync.dma_start(cb_sb[:], CB.ap()[:])
                nc.sync.dma_start(wt_sb[:, 6 * 128:wt_bounds[1] * 128],
                                  WT.ap()[:, 6 * 128:wt_bounds[1] * 128])
            for c in range(0 if bg else 1, NCH):
                nc.sync.dma_start(x_sb[:, c * NB * LW:(c + 1) * NB * LW],
                                  X.ap()[bg, c])
            wm_sb = mpool.tile([128, 3 * NB * SL], bf16, tag="wm",
                               name=f"wm_sb_{bg}")
            nc.sync.dma_start(wm_sb[:], WM.ap()[bg])
            if bg == 0:
                # stream the rest of the weights in behind x/wm for bg0
                for wi in range(1, len(wt_bounds) - 1):
                    nc.sync.dma_start(
                        wt_sb[:, wt_bounds[wi] * 128:wt_bounds[wi + 1] * 128],
                        WT.ap()[:, wt_bounds[wi] * 128:wt_bounds[wi + 1] * 128])
                nc.sync.dma_start(fcw_sb[:], FCW.ap()[:])
                for j in range(5):
                    nc.sync.dma_start(
                        feat32[:, (18 + j) * BC:(19 + j) * BC],
                        E12.ap()[j * 128:(j + 1) * 128, :])
            xv = x_sb[:].rearrange("p (c b w) -> p c b w", c=NCH, b=NB)

            def pool_quad(bg, ki, h, q, ps, fused=False):
                th = tpool.tile([128, NF], bf16, tag="th",
                                name=f"th_{bg}_{ki}_{h}_{q}")
                nc.scalar.activation(
                    th[:], ps[:], AF.Tanh,
                    bias=cb_sb[:, ki * 2 + h:ki * 2 + h + 1])
                for seg in range(3):
                    ch = ki * 6 + seg * 2 + h
                    col0 = ch * BC + bg * NB + q * 4
                    wmq = wm_sb[:, seg * NB * SL + q * NF:
                                seg * NB * SL + (q + 1) * NF]
                    pr = prpool.tile([128, NF], bf16, tag="pr",
                                     name=f"pr_{bg}_{ki}_{h}_{q}_{seg}")
                    nc.vector.tensor_tensor(pr[:], th[:], wmq,
                                            op=ALU.mult)
                    nc.vector.tensor_reduce(
                        feat32[:, col0:col0 + 4],
                        pr[:].rearrange("p (b w) -> p b w", w=SL),
                        axis=AX.X, op=ALU.add)

            def conv_group(bg, ki, h, q_outer):
                k = KS[ki]
                nsets = k * NCH
                # c-major: early sets touch only early x chunks, so the
                # cold-start conv isn't gated on the full x DMA
                sets = [(t, c) for c in range(NCH) for t in range(k)]
                pss = [pspool.tile([128, NF], f32, tag="ps",
                                   name=f"ps_{bg}_{ki}_{h}_{q}")
                       for q in range(NQ)]
                if q_outer:
                    # staggered: each quad's matmuls complete in turn so
                    # pooling (and the feature cast) overlaps the remaining
                    # quads' matmuls
                    for q in range(NQ):
                        for si, (t, c) in enumerate(sets):
                            s = t - k // 2
                            bi = _bidx(ki, h, t, c)
                            rhs = xv[:, c, q * 4:(q + 1) * 4,
                                     PAD + s:PAD + s + SL]
                            nc.tensor.matmul(pss[q][:],
                                             wt_sb[:, bi * 128:(bi + 1) * 128],
                                             rhs, start=(si == 0),
                                             stop=(si == nsets - 1))
                        pool_quad(bg, ki, h, q, pss[q])
                        cast_cols(bg * NB + q * 4, 4)
                else:
                    for si, (t, c) in enumerate(sets):
                        s = t - k // 2
                        bi = _bidx(ki, h, t, c)
                        wblk = wt_sb[:, bi * 128:(bi + 1) * 128]
                        for q in range(NQ):
                            rhs = xv[:, c, q * 4:(q + 1) * 4,
                                     PAD + s:PAD + s + SL]
                            nc.tensor.matmul(pss[q][:], wblk, rhs,
                                             start=(si == 0),
                                             stop=(si == nsets - 1))
                    for q in range(NQ):
                        pool_quad(bg, ki, h, q, pss[q])

            def cast_cols(b0, nb):
                nc.vector.tensor_copy(
                    featbf[:].rearrange("p (c b) -> p c b",
                                        c=NCHK)[:, :, b0:b0 + nb],
                    feat32[:].rearrange("p (c b) -> p c b",
                                        c=NCHK)[:, :, b0:b0 + nb])

            def fc_half(half):
                b0 = half * 32
                for ch in range(NCHK):
                    nc.tensor.matmul(
                        fcps[b0:b0 + 32, :],
                        featbf[:, ch * BC + b0:ch * BC + b0 + 32],
                        fcw_sb[:, ch * VR:(ch + 1) * VR],
                        start=(ch == 0), stop=(ch == NCHK - 1))

            for gi, (ki, h) in enumerate([(ki, h) for ki in range(3)
                                          for h in range(2)]):
                final = (bg == NBG - 1) and (ki, h) == (2, 1)
                conv_group(bg, ki, h, q_outer=final)
                if gi == 0 and bg == 2:
                    # batch half 0 fully pooled at end of bg1; cast dep is
                    # ready by now so no PE bubble
                    cast_cols(0, 32)
                    fc_half(0)
                if gi == 0 and bg == 3:
                    cast_cols(32, 16)  # b 32..47 pooled at end of bg2

        fc_half(1)  # b 48..63 already cast per-quad in the final group
        mx = const.tile([BC, 1], f32)
        nc.vector.tensor_reduce(mx[:], fcps[:], axis=AX.X, op=ALU.max,
                                negate=True)
        esm = const.tile([BC, VR], f32)
        ssum = const.tile([BC, 1], f32)
        nc.scalar.activation(esm[:], fcps[:], AF.Exp, bias=mx[:],
                             accum_out=ssum[:])
        rin = const.tile([BC, 1], f32)
        nc.vector.reciprocal(rin[:], ssum[:])
        osb = const.tile([BC, VR], f32)
        nc.vector.tensor_scalar_mul(osb[:], esm[:], rin[:])
        nc.sync.dma_start(OUT.ap()[:], osb[:])

    nc.compile()
    return nc


_NC_CACHE = []


def _get_program():
    if not _NC_CACHE:
        _NC_CACHE.append(_build_program())
    return _NC_CACHE[0]


def _prep_inputs(W, e1, e2, pos_emb1, pos_emb2, conv_ws, conv_bs, fc_w, fc_b,
                 W_pos1, W_pos2, e1_p, e2_p):
    """Host-side data layout; returns per-core input maps."""
    # --- conv input: [B, 1024, 128] zero-padded, 3 zero cols each side ---
    Wp1 = pos_emb1[W_pos1]          # [B, S, DP]
    Wp2 = pos_emb2[W_pos2]
    Xf = np.concatenate([W, Wp1, Wp2], axis=2).transpose(0, 2, 1)  # [B,CIN,S]
    Xpad = np.zeros((B, CINP, LW), np.float32)
    Xpad[:, :CIN, PAD:PAD + S] = Xf
    Xpad = Xpad.astype(BF16).reshape(NCORE, NBG, NB, NCH, 128, LW)
    Xc = np.ascontiguousarray(Xpad.transpose(0, 1, 3, 4, 2, 5)).reshape(
        NCORE, NBG, NCH, 128, NB * LW)

    # --- segment weight masks m/cnt, replicated over 128 partitions ---
    d1 = np.minimum(e1_p, e2_p).astype(np.int64)
    d2 = np.maximum(e1_p, e2_p).astype(np.int64)
    idx = np.arange(S)[None, :]
    m1 = (idx < d1[:, None])
    m2 = (idx >= d1[:, None]) & (idx < d2[:, None])
    m3 = (idx >= d2[:, None]) & (idx < S - 1)
    wm = np.stack([m1, m2, m3], axis=1).astype(np.float32)  # [B,3,S]
    cnt = np.maximum(wm.sum(axis=2), 1.0)
    wm /= cnt[:, :, None]
    wm = wm[:, :, :SL]  # l=127 is never pooled
    wm = wm.astype(BF16).reshape(NCORE, NBG, NB, 3, SL)
    wm = np.ascontiguousarray(wm.transpose(0, 1, 3, 2, 4)).reshape(
        NCORE, NBG, 1, 3 * NB * SL)
    WMc = np.ascontiguousarray(np.broadcast_to(
        wm, (NCORE, NBG, 128, 3 * NB * SL)))

    # --- conv weights -> stationary blocks [128ci, 128co], bf16 ---
    wt = np.zeros((128, NSETS * 128), np.float32)
    for ki, k in enumerate(KS):
        cw = np.zeros((DC, CINP, k), np.float32)
        cw[:, :CIN, :] = conv_ws[ki]
        for h in range(2):
            for t in range(k):
                for c in range(NCH):
                    blk = cw[h * 128:(h + 1) * 128,
                             c * 128:(c + 1) * 128, t]  # [co, ci]
                    wt[:, _bidx(ki, h, t, c) * 128:
                       (_bidx(ki, h, t, c) + 1) * 128] = blk.T
    wt = wt.astype(BF16)

    # --- fc weights in device feature order; fc_b via constant-1 feature ---
    # f' in [0, 2304): ch = ki*6+seg*2+h, p = co_local
    #   orig col = 600 + ki*768 + (h*128+p)*3 + seg
    # f' in [2304, 2904): orig col = f' - 2304   (e1, e2)
    # f' == 2904: constant-1 -> fc_b
    fcw = np.zeros((NCHK * 128, VR), np.float32)
    fp = np.arange(F_CONV)
    ch = fp // 128
    p = fp % 128
    ki = ch // 6
    seg = (ch % 6) // 2
    h = ch % 2
    orig = 600 + ki * 768 + (h * 128 + p) * 3 + seg
    fcw[fp] = fc_w[:, orig].T
    fcw[F_CONV:F_CONV + 600] = fc_w[:, :600].T
    fcw[F_CONV + 600] = fc_b
    fcw_host = np.ascontiguousarray(
        fcw.reshape(NCHK, 128, VR).transpose(1, 0, 2)).reshape(
        128, NCHK * VR).astype(BF16)

    # --- e1/e2 + constant-1 features, fp32, per core [640, BC] ---
    e12 = np.zeros((B, 5 * 128), np.float32)
    e12[:, :300] = e1
    e12[:, 300:600] = e2
    e12[:, 600] = 1.0
    E12c = np.ascontiguousarray(
        e12.reshape(NCORE, BC, 5 * 128).transpose(0, 2, 1))

    # --- conv biases [128, 6] fp32 ---
    cb = np.zeros((128, 6), np.float32)
    for ki in range(3):
        for h in range(2):
            cb[:, ki * 2 + h] = conv_bs[ki][h * 128:(h + 1) * 128]

    in_maps = []
    for i in range(NCORE):
        in_maps.append({
            "X": Xc[i], "WM": WMc[i], "WT": wt, "FCW": fcw_host,
            "E12": E12c[i], "CB": cb,
        })
    return in_maps


def kernel(**inputs):
    f = {k: np.asarray(v) for k, v in inputs.items()}
    in_maps = _prep_inputs(
        f["W"].astype(np.float32), f["e1"].astype(np.float32),
        f["e2"].astype(np.float32), f["pos_emb1"].astype(np.float32),
        f["pos_emb2"].astype(np.float32),
        [f["conv_w3"], f["conv_w5"], f["conv_w7"]],
        [f["conv_b3"], f["conv_b5"], f["conv_b7"]],
        f["fc_w"].astype(np.float32), f["fc_b"].astype(np.float32),
        f["W_pos1"], f["W_pos2"], f["e1_p"], f["e2_p"])

    from concourse.bass_utils import run_bass_kernel_spmd
    nc = _get_program()
    try:
        res = run_bass_kernel_spmd(nc, in_maps, core_ids=list(range(NCORE)))
    except Exception:
        # device wedge (e.g. NRT_EXEC_UNIT_UNRECOVERABLE left by a prior
        # crashed process): reset the runtime, then retry once
        try:
            import ctypes
            import jax
            jax.devices()
            ctypes.CDLL("/opt/axon/libaxon_pjrt.so").axon_reset()
        except Exception:
            pass
        res = run_bass_kernel_spmd(nc, in_maps, core_ids=list(range(NCORE)))
    out = np.concatenate([res.results[i]["OUT"] for i in range(NCORE)],
                         axis=0)
    return out.astype(np.float32)
